# revision 27
# baseline (speedup 1.0000x reference)
"""Self-contained Trainium2 Bass kernel for nn_Decoder_79809082294812.

kernel(**inputs) takes the FULL unsharded inputs (embeddings [1024,1000,128],
remaining_capacity [1024], Wqg [257,128], Wkg/Wvg/Wog/Wqo/Wko [128,128],
current_node [1024], mask [1024,1000]) and returns (probs, logits), each
[1024, 1000] float32 — matching the reference decoder.

Sharding: pure data-parallel over the batch dim across 8 NeuronCores
(128 batch elements per core); weights replicated.

Device pipeline (per core, 8 tiles of 16 batch elements):
  - host precomputes q = context@Wqg and the per-element U matrices
    (U_b = (Wkg/sqrt(D)) @ q_b per head), packed as U32 [E, b, 32] with
    U at column offset 8*(b%4), so 16 elements' compat rows pack densely
    into one [128, 1024] PSUM tile (rows 32*(j//4) + 8*(j%4) + h) via
    accumulating matmuls at 4 tile_positions.
  - softmax without max-subtraction (|compat| < ~8), normalization folded
    into the attention transpose: attnT = exN^T @ diag(recip) as a regular
    matmul with a runtime diagonal moving operand.
  - glimpse accumulation A streams the natural-layout chunks as matmul
    stationaries, interleaved with the next tile's compat matmuls so the
    128-column LDWEIGHTS hide behind 512-column moving matmuls.
  - comp rows for all 128 batch elements accumulate into a single dense
    [128, 1024] PSUM tile (stationary w at column b%32, tile_position
    32*(b//32)), so the tanh/softmax epilogue is 3 dense [128,1000] passes.
  - logits are output as tanh(comp); the *10 scale is applied on host.
"""
import contextlib
import ctypes
import math
import os
import sys
import types

sys.path.insert(0, '/opt/trn_rl_repo')

from contextlib import ExitStack
import numpy as np
import ml_dtypes

import concourse.bass as bass
import concourse.tile as tile
from concourse import bacc, mybir
from concourse.bass_utils import run_bass_kernel_spmd

F32 = mybir.dt.float32
BF16 = mybir.dt.bfloat16
FP8 = mybir.dt.float8e4
AF = mybir.ActivationFunctionType
AX = mybir.AxisListType
ALU = mybir.AluOpType
BF16_NP = ml_dtypes.bfloat16
FP8_NP = ml_dtypes.float8_e4m3fn

B = 1024
N = 1000
E = 128
H = 8
D = 16
N_CORES = 8
BC = B // N_CORES   # batch elements per core
TB = 16             # batch elements per tile
NT = BC // TB       # tiles per core
NCH = 8             # n-chunks (node n lives at chunk n%8, row n//8)
CH = 125            # rows per chunk
SPLIT = 512         # psum-bank-aligned split of the n axis

NAT_FP8 = False     # natural-layout embedding stream dtype (fp8 halves DMA)

WNAME_SHAPES = {
    "wvg": ([E, E], BF16),
    "wbig": ([E, E], BF16),
    "m8rep": ([E, TB * H], F32),
    "identf": ([128, 128], F32),
}

_NC_CACHE = {}
LAST_RESULT = None   # BassKernelResults of the most recent run (for profiling)


# --------------------------------------------------------------------------
# Optional NTFF profiling hook (enabled only when BASS_TRACE is set).
# --------------------------------------------------------------------------
def _install_profile_shim():
    so_path = '/opt/axon/libaxon_pjrt.so'
    try:
        import antenv
    except ImportError:
        return
    if 'antenv.axon_hooks' not in sys.modules:
        mod = types.ModuleType('antenv.axon_hooks')
        mod._hook = None

        def set_axon_ntff_profile_hook(h):
            mod._hook = h

        def get_axon_ntff_profile_hook():
            return mod._hook

        mod.set_axon_ntff_profile_hook = set_axon_ntff_profile_hook
        mod.get_axon_ntff_profile_hook = get_axon_ntff_profile_hook
        sys.modules['antenv.axon_hooks'] = mod
        antenv.axon_hooks = mod
    mod = sys.modules['antenv.axon_hooks']
    if mod.get_axon_ntff_profile_hook() is not None:
        return
    try:
        lib = ctypes.CDLL(so_path)
    except OSError:
        return
    if not hasattr(lib, "axon_start_nrt_profile"):
        return
    lib.axon_start_nrt_profile.argtypes = [ctypes.POINTER(ctypes.c_int64),
                                           ctypes.c_size_t]
    lib.axon_start_nrt_profile.restype = ctypes.c_int64
    lib.axon_stop_nrt_profile.argtypes = [ctypes.c_char_p]
    lib.axon_stop_nrt_profile.restype = ctypes.c_int64

    @contextlib.contextmanager
    def _hook(output_dir, device_ids):
        import jax
        jax.devices()
        if device_ids:
            ids = (ctypes.c_int64 * len(device_ids))(*device_ids)
            rc = lib.axon_start_nrt_profile(ids, len(device_ids))
        else:
            rc = lib.axon_start_nrt_profile(None, 0)
        if rc != 0:
            raise RuntimeError(f"axon_start_nrt_profile rc={rc}")
        try:
            yield
        finally:
            n = lib.axon_stop_nrt_profile(str(output_dir).encode())
            if n < 0:
                raise RuntimeError(f"axon_stop_nrt_profile rc={n}")

    mod.set_axon_ntff_profile_hook(_hook)
    import concourse.bass_utils as bu
    bu.upload_artifacts = lambda tmpdir: f"local:{tmpdir}"


def _host_prep_weights(Wvg, Wog, Wqo, Wko):
    w = {}
    w["wvg"] = np.ascontiguousarray(Wvg.astype(BF16_NP))
    w["wbig"] = np.ascontiguousarray(
        ((Wog @ Wqo @ Wko.T) / math.sqrt(E)).astype(BF16_NP))
    # head-extraction mask over (j, h) lanes: [hd, j*H + h] = (hd//D == h)
    m8 = np.zeros((E, H), np.float32)
    for hd in range(E):
        m8[hd, hd // D] = 1.0
    w["m8rep"] = np.ascontiguousarray(np.tile(m8, (1, TB)))
    w["identf"] = np.eye(128, dtype=np.float32)
    return w


def _host_prep_u32(embeddings, remaining_capacity, Wqg, Wkg, current_node):
    """U32 [E, B, 32] bf16: U for element b at columns 8*(b%4)..+8."""
    graph = embeddings.mean(axis=1)                       # [B, E]
    cur = embeddings[np.arange(B), current_node]          # [B, E]
    context = np.concatenate(
        [graph, cur, remaining_capacity[:, None]], axis=-1)
    q = (context @ Wqg).reshape(B, H, D)
    U = np.einsum('ehd,bhd->ebh',
                  (Wkg / math.sqrt(D)).reshape(E, H, D).astype(np.float32),
                  q.astype(np.float32))                   # [E, B, H]
    U32 = np.zeros((E, B, 32), dtype=BF16_NP)
    off = 8 * (np.arange(B) % 4)
    for r in range(4):
        sel = off == 8 * r
        U32[:, sel, 8 * r:8 * r + 8] = U[:, sel, :].astype(BF16_NP)
    return U32, cur


def _build_nc(Bc=BC, n_devices=N_CORES):
    nat_dt, nat_np_elem = (FP8, 1) if NAT_FP8 else (BF16, 2)
    nc = bacc.Bacc("TRN2", target_bir_lowering=False, debug=False,
                   num_devices=n_devices)

    # embT comes from an xbar DMA-transpose of the natural layout (giant
    # contiguous DRAM reads; DGE descriptor emission was the bottleneck
    # with per-partition-line descriptors).  nat keeps the host-transposed
    # per-partition-contiguous layout, split across the scalar+gpsimd
    # queues so it runs parallel to the xbar stream on sync.
    embd = nc.dram_tensor("emb", [Bc, N, E], BF16, kind="ExternalInput").ap()
    natd = nc.dram_tensor("nat", [CH, Bc, NCH, E], nat_dt,
                          kind="ExternalInput").ap()
    u32d = nc.dram_tensor("u32", [E, Bc * 32], BF16, kind="ExternalInput").ap()
    wap = {k: nc.dram_tensor(k, s, dt, kind="ExternalInput").ap()
           for k, (s, dt) in WNAME_SHAPES.items()}
    probs_out = nc.dram_tensor("probs", [Bc, N], F32, kind="ExternalOutput").ap()
    tanh_out = nc.dram_tensor("tanh", [Bc, N], F32, kind="ExternalOutput").ap()

    with tile.TileContext(nc) as tc, ExitStack() as ctx:
        # ---- constants ----
        cpool = ctx.enter_context(tc.tile_pool(name="consts", bufs=1))
        w_sb = {}
        for k, (s, dt) in WNAME_SHAPES.items():
            t = cpool.tile(s, dt, tag=k)
            nc.scalar.dma_start(t[:], wap[k][:])
            w_sb[k] = t
        u32_sb = cpool.tile([E, Bc, 32], BF16, tag="u32")
        nc.scalar.dma_start(
            u32_sb[:], u32d[:].rearrange("e (b k) -> e b k", k=32))

        # ---- pools ----
        embT_pool = ctx.enter_context(tc.tile_pool(name="embT", bufs=2))
        nat_pool = ctx.enter_context(tc.tile_pool(name="nat", bufs=2))
        exn_pool = ctx.enter_context(tc.tile_pool(name="exn", bufs=2))
        attnT_pool = ctx.enter_context(tc.tile_pool(name="attnT", bufs=2))
        sm_pool = ctx.enter_context(tc.tile_pool(name="smalls", bufs=2))
        stage_pool = ctx.enter_context(tc.tile_pool(name="stage", bufs=1))

        # PSUM (8 banks): pcm 2x2 + pcomp 1x2 + pat 1x1 + paux 1x1
        pcm_pool = ctx.enter_context(tc.tile_pool(name="pcm", bufs=2, space="PSUM"))
        pcomp_pool = ctx.enter_context(tc.tile_pool(name="pcomp", bufs=1, space="PSUM"))
        pat_pool = ctx.enter_context(tc.tile_pool(name="pat", bufs=1, space="PSUM"))
        paux_pool = ctx.enter_context(tc.tile_pool(name="paux", bufs=1, space="PSUM"))

        pcomp = pcomp_pool.tile([128, 1024], F32, tag="pcomp")
        t_th = stage_pool.tile([128, N], F32, tag="tanh")
        p_stage = stage_pool.tile([128, N], F32, tag="probs")

        # per-tile state carried across the software pipeline
        state = [None] * NT  # (embTg, natg, exn, diagb)

        def load_tile(t):
            embTg = embT_pool.tile([E, TB, N], BF16, tag="embT")
            nc.sync.dma_start_transpose(
                embTg[:],
                embd[t * TB:(t + 1) * TB].rearrange("b n e -> (b n) e"))
            natg = nat_pool.tile([CH, TB, NCH, E], nat_dt, tag="nat")
            # NOTE: concurrent xbar transposes on different HWDGE rings
            # corrupt each other -- only sync may issue dma_start_transpose.
            nsp = 60
            nc.scalar.dma_start(natg[0:nsp], natd[0:nsp, t * TB:(t + 1) * TB])
            nc.gpsimd.dma_start(natg[nsp:CH], natd[nsp:CH, t * TB:(t + 1) * TB])
            return embTg, natg

        def compat_mms(t, embTg, a_pairs):
            """Issue the 32 compat matmuls for tile t, interleaving the
            previous tile's A-pass (LDW+small-MM) pairs between them."""
            pcm = pcm_pool.tile([128, 1024], F32, tag="pcm")
            k = 0
            ap_idx = 0
            for q in range(4):
                for pp in range(4):
                    j = 4 * pp + q
                    for s0, s1 in ((0, SPLIT), (SPLIT, N)):
                        nc.tensor.matmul(
                            pcm[32 * pp:32 * pp + 32, s0:s1],
                            u32_sb[:, t * TB + j, :],
                            embTg[:, j, s0:s1],
                            start=(q == 0), stop=(q == 3),
                            tile_position=(0, 32 * pp))
                        # interleave 4 A-pairs behind each 512-col matmul
                        for _ in range(4):
                            if ap_idx < len(a_pairs):
                                a_pairs[ap_idx]()
                                ap_idx += 1
                    k += 1
            while ap_idx < len(a_pairs):
                a_pairs[ap_idx]()
                ap_idx += 1
            return pcm

        def softmax_tile(t, pcm):
            exn = exn_pool.tile([128, N], BF16, tag="exn")
            sums = sm_pool.tile([128, 1], F32, tag="sums")
            nc.scalar.activation(exn[:], pcm[:, :N], AF.Exp, accum_out=sums[:])
            recip = sm_pool.tile([128, 1], F32, tag="recip")
            nc.vector.reciprocal(recip[:], sums[:])
            diagb = sm_pool.tile([128, 128], BF16, tag="diagb")
            nc.vector.tensor_scalar_mul(diagb[:], w_sb["identf"][:], recip[:])
            return exn, diagb

        def transpose_tile(t, exn, diagb):
            """attnT [125, c, 128] bf16: normalized attn, node 8p+c at row p."""
            attnT = attnT_pool.tile([CH, NCH, 128], BF16, tag="attnT")
            exn_v = exn[:].rearrange("r (p c) -> r c p", c=NCH)
            for half in range(2):
                pat = pat_pool.tile([CH, 4, 128], F32, tag="pat")
                for cc in range(4):
                    c = 4 * half + cc
                    nc.tensor.matmul(pat[:, cc, :], exn_v[:, c, :], diagb[:],
                                     start=True, stop=True)
                nc.scalar.copy(attnT[:, 4 * half:4 * half + 4, :], pat[:])
            return attnT

        def a_pass_pairs(t, natg, attnT, pA):
            """List of thunks, each issuing one (LDW nat-chunk, 8-col MM)."""
            pairs = []
            for j in range(TB):
                col0 = 32 * (j // 4) + 8 * (j % 4)
                for c in range(NCH):
                    def mk(j=j, c=c, col0=col0):
                        nc.tensor.matmul(
                            pA[:, j * H:(j + 1) * H],
                            natg[:, j, c, :],
                            attnT[:, c, col0:col0 + H],
                            start=(c == 0), stop=(c == NCH - 1))
                    pairs.append(mk)
            return pairs

        def heads_w_comp(t, paux, pA, embTg):
            A_sb = sm_pool.tile([E, TB * H], BF16, tag="A")
            nc.scalar.copy(A_sb[:], pA)
            pheads = paux[:, 128:256]
            nc.tensor.matmul(pheads, w_sb["wvg"][:], A_sb[:],
                             start=True, stop=True)
            tmp = sm_pool.tile([E, TB * H], F32, tag="tmp")
            nc.vector.tensor_mul(tmp[:], pheads, w_sb["m8rep"][:])
            heads = sm_pool.tile([E, TB], BF16, tag="heads")
            with nc.allow_low_precision(reason="heads ~O(1); bf16 validated"):
                nc.vector.reduce_sum(
                    heads[:], tmp[:].rearrange("p (j h) -> p j h", h=H),
                    axis=AX.X)
            pw = paux[:, 256:272]
            nc.tensor.matmul(pw, w_sb["wbig"][:], heads[:],
                             start=True, stop=True)
            # w32g [128, j, 32] with w for element b=t*TB+j at column b%32
            w32g = sm_pool.tile([E, TB, 32], BF16, tag="w32g")
            nc.gpsimd.memset(w32g[:], 0.0)
            base = w32g[:]
            dst = bass.AP(base.tensor,
                          base.offset + 16 * (t % 2),
                          [list(base.ap[0]), [33, TB]])
            nc.scalar.copy(dst, pw)
            # comp matmuls: accumulate into the core-wide dense pcomp
            for j in range(TB):
                b = t * TB + j
                ppc = b // 32
                r = b % 32
                for s0, s1 in ((0, SPLIT), (SPLIT, N)):
                    nc.tensor.matmul(
                        pcomp[32 * ppc:32 * ppc + 32, s0:s1],
                        w32g[:, j, :],
                        embTg[:, j, s0:s1],
                        start=(r == 0), stop=(r == 31),
                        tile_position=(0, 32 * ppc))

        # -------- software-pipelined main loop --------
        # iteration i issues: transposes(i-1), compat(i) ⊗ A(i-1),
        # heads/w/comp(i-1), softmax prep for tile i.
        prev = None   # (natg, attnT, embTg_prev)
        pcm_prev = None
        for i in range(NT + 1):
            if i < NT:
                embTg, natg = load_tile(i)
            a_pairs = []
            if prev is not None:
                natg_p, exn_p, diagb_p, embTg_p = prev
                attnT_p = transpose_tile(i - 1, exn_p, diagb_p)
                paux = paux_pool.tile([E, 512], F32, tag="paux")
                pA = paux[:, 0:128]
                a_pairs = a_pass_pairs(i - 1, natg_p, attnT_p, pA)
            if i < NT:
                pcm = compat_mms(i, embTg, a_pairs)
            else:
                for f in a_pairs:
                    f()
            if prev is not None:
                heads_w_comp(i - 1, paux, pA, embTg_p)
            if i < NT:
                exn, diagb = softmax_tile(i, pcm)
                prev = (natg, exn, diagb, embTg)

        # -------- epilogue: tanh, probs softmax (no-max), outputs --------
        nc.scalar.activation(t_th[:], pcomp[:, :N], AF.Tanh)
        nc.gpsimd.dma_start(tanh_out[:], t_th[:])
        sums2 = stage_pool.tile([128, 1], F32, tag="sums2")
        nc.scalar.activation(p_stage[:], t_th[:], AF.Exp, scale=10.0,
                             accum_out=sums2[:])
        recip2 = stage_pool.tile([128, 1], F32, tag="recip2")
        nc.vector.reciprocal(recip2[:], sums2[:])
        nc.vector.tensor_scalar_mul(p_stage[:], p_stage[:], recip2[:])
        nc.sync.dma_start(probs_out[:], p_stage[:])

    nc.compile()
    return nc


def _get_nc():
    key = (BC, N_CORES)
    if key not in _NC_CACHE:
        _NC_CACHE[key] = _build_nc(*key)
    return _NC_CACHE[key]


def kernel(embeddings, remaining_capacity, Wqg, Wkg, Wvg, Wog, Wqo, Wko,
           current_node, mask):
    global LAST_RESULT
    embeddings = np.asarray(embeddings, dtype=np.float32)
    remaining_capacity = np.asarray(remaining_capacity, dtype=np.float32)
    Wqg = np.asarray(Wqg, dtype=np.float32)
    Wkg = np.asarray(Wkg, dtype=np.float32)
    Wvg = np.asarray(Wvg, dtype=np.float32)
    Wog = np.asarray(Wog, dtype=np.float32)
    Wqo = np.asarray(Wqo, dtype=np.float32)
    Wko = np.asarray(Wko, dtype=np.float32)
    current_node = np.asarray(current_node).astype(np.int64)
    mask = np.asarray(mask)
    assert embeddings.shape == (B, N, E)

    trace = bool(os.environ.get("BASS_TRACE"))
    if trace:
        _install_profile_shim()

    w = _host_prep_weights(Wvg, Wog, Wqo, Wko)
    U32, cur = _host_prep_u32(embeddings, remaining_capacity, Wqg, Wkg,
                              current_node)
    emb_bf = embeddings.astype(BF16_NP)                        # [B, N, E]
    # nat [125, B, 8, E]: nat[p,b,c,e] = emb[b, 8p+c, e]; per-core slices
    # give one contiguous 32KB DRAM run per (partition, tile).
    nat_t = np.ascontiguousarray(
        emb_bf.reshape(B, CH, NCH, E).transpose(1, 0, 2, 3))

    nc = _get_nc()
    in_maps = []
    for c in range(N_CORES):
        sl = slice(c * BC, (c + 1) * BC)
        m = {
            "emb": emb_bf[sl],
            "nat": np.ascontiguousarray(nat_t[:, sl]),
            "u32": np.ascontiguousarray(U32[:, sl].reshape(E, BC * 32)),
        }
        m.update(w)
        in_maps.append(m)

    kw = {}
    if trace:
        kw = dict(trace=True, trace_cores=[0])
    res = run_bass_kernel_spmd(nc, in_maps, list(range(N_CORES)), **kw)
    LAST_RESULT = res

    probs = np.concatenate([res.results[c]["probs"] for c in range(N_CORES)], 0)
    tanh = np.concatenate([res.results[c]["tanh"] for c in range(N_CORES)], 0)
    logits = 10.0 * tanh

    if mask.any():
        # General-correctness slow path (the spec always sends an all-False
        # mask): the mask affects the glimpse attention too, so recompute
        # everything for the masked rows on the host.
        probs, logits = _numpy_full(embeddings, remaining_capacity, Wqg, Wkg,
                                    Wvg, Wog, Wqo, Wko, cur, mask)

    return probs.astype(np.float32), logits.astype(np.float32)


def _numpy_full(emb, capv, Wqg, Wkg, Wvg, Wog, Wqo, Wko, cur, mask):
    graph = emb.mean(axis=1)
    context = np.concatenate([graph, cur, capv[:, None]], axis=-1)
    q = (context @ Wqg).reshape(B, H, D)
    k = (emb @ Wkg).reshape(B, N, H, D)
    v = (emb @ Wvg).reshape(B, N, H, D)
    compat = np.einsum('bhd,bnhd->bhn', q, k) / math.sqrt(D)
    compat = np.where(mask[:, None, :], -np.inf, compat)
    m = compat.max(axis=-1, keepdims=True)
    a = np.exp(compat - m)
    attn = a / a.sum(axis=-1, keepdims=True)
    heads = np.einsum('bhn,bnhd->bhd', attn, v).reshape(B, E)
    glimpse = heads @ Wog
    qo = glimpse @ Wqo
    ko = emb @ Wko
    comp = np.einsum('be,bne->bn', qo, ko) / math.sqrt(E)
    logits = 10.0 * np.tanh(comp)
    logits = np.where(mask, -np.inf, logits)
    m2 = logits.max(axis=-1, keepdims=True)
    a2 = np.exp(logits - m2)
    probs = a2 / a2.sum(axis=-1, keepdims=True)
    return probs.astype(np.float32), logits.astype(np.float32)
